# revision 2
# baseline (speedup 1.0000x reference)
"""Trainium2 Bass kernel: cosine-similarity message passing (GNN aggregate).

Math (collapsed — the [N,N] similarity matrix is never materialized):
    x_hat = x / max(||x||, eps)                      row-normalized features
    G'    = x_hat.T @ [x | 1]        [D, D+1]        Gram + column-sum s
    oa    = x @ G'                   [N, D+1]        (query-side normalization
                                                      cancels in the ratio)
    out   = oa[:, :D] / oa[:, D:D+1]
Precision: everything fp32. fp32r (=TF32) matmuls were measured SLOWER
in situ (extra rounding passes outweigh the 2x PE matmul speedup), and
any reduced precision on the s/row_sum path explodes the error on
near-cancelling rows (|row_sum| ~ 1e-3 rows dominate the fro norm).

Sharding: each core loads ONLY its own N/8 x D row block (1 MB),
computes the partial Gram over its rows, and an 8-core AllReduce of the
263 KB G' produces the full Gram everywhere. The x_own transposes for
phase 2 run on PE during the AllReduce wait.

v7 (this version): instruction-count diet of the previous baseline —
measured ~5% faster in interleaved A/B (26.6us vs 28.0us steady-state
per-iter; environment drifts +-25% between minutes, so only interleaved
comparisons are trustworthy):
 - norms: all 8 row-tiles via ScalarE Square+accum (drops the DVE
   bn_stats/bn_aggr path: -10 DVE ops), one batched Sqrt.
 - scale x_hat = x * (1/norm): ONE batched DVE multiply.
 - phase 2: matmuls into 2-bank PSUM groups; 4 grouped PSUM->SBUF
   copies alternating ScalarE/DVE; ONE strided batched reciprocal of
   the 8 row-sums; two half-batched multiplies so the output DMA of
   the first half overlaps the second.
 - AllReduce in-situ marginal cost is only ~3.4us in the pipelined
   steady state (latency ~13us is hidden by the depth-2 software
   pipeline); payload-shrink tricks (symmetric-half packing, bf16)
   are not worth their unpack cost / precision risk.

The iters>1 builds (used by the steady-state delta timing) are
software-pipelined depth 2: phase 2 of iter i is emitted after the cc
launch of iter i+2, so no engine queue ever blocks on an in-flight
AllReduce.

Environment quirks:
 - this walrus build accepts at most ONE sync wait per instruction:
   _legalize_sync_waits hoists extras onto same-engine Drain carriers.
 - Rsqrt/Reciprocal activation funcs are disabled in bass (accuracy);
   norm recip = ACT Sqrt + DVE reciprocal.
 - eps in max(||x||, eps) never binds for gaussian rows (min norm ~14).
 - DRAM-bounce collectives (SBUF collectives are disabled in bass), all
   cc-adjacent DMAs on the gpsimd queue for straight-line ordering.
"""

import numpy as np
from contextlib import ExitStack

import concourse.bass as bass
import concourse.tile as tile
from concourse import mybir
from concourse.masks import make_identity
from concourse.bass_utils import run_bass_kernel_spmd

N, D = 8192, 256
NCORES = 8
P = 128
OWN = N // NCORES            # 1024 rows per core
OWN_T = OWN // P             # 8 own tiles
DA = D + 1                   # 257: x columns + ones column
F32 = mybir.dt.float32
AF = mybir.ActivationFunctionType

_nc_cache = {}


def _legalize_sync_waits(bir_bytes: bytes) -> bytes:
    """This walrus build accepts at most ONE sync wait per instruction.
    Tile emits several; hoist the extras onto same-engine Drain
    instructions placed immediately before (queue order preserves the
    semantics of inline waits)."""
    import orjson
    bir = orjson.loads(bir_bytes)
    ctr = [0]

    def fix_block(blk):
        new_list = []
        for ins in blk.get("instructions", []):
            si = ins.get("sync_info")
            if si:
                waits = si.get("on_wait") or []
                if len(waits) > 1:
                    for w in waits[:-1]:
                        ctr[0] += 1
                        new_list.append({
                            "debug": ins.get("debug", 0),
                            "engine": ins["engine"],
                            "ins": [], "outs": [],
                            "name": f"I-lw{ctr[0]}",
                            "opcode": "Drain",
                            "sync_info": {"on_update": [], "on_wait": [w]},
                        })
                    si["on_wait"] = waits[-1:]
            new_list.append(ins)
        blk["instructions"] = new_list
        for sb in blk.get("blocks", []):
            fix_block(sb)

    for f in bir["functions"]:
        for blk in f["blocks"]:
            fix_block(blk)
    return orjson.dumps(bir)


def _build_nc(iters: int = 1):
    nc = bass.Bass(
        "TRN2", target_bir_lowering=False, debug=False, enable_asserts=True,
        num_devices=NCORES,
    )
    x_own = nc.declare_dram_parameter("x_own", [OWN, D], F32, isOutput=False)
    out = nc.declare_dram_parameter("out", [OWN, D], F32, isOutput=True)

    # row order: row = p*8 + t  -> 8 KB contiguous HBM reads per partition
    xo = x_own.ap().rearrange("(p t) d -> p t d", p=P)
    ov = out.ap().rearrange("(p t) d -> p t d", p=P)

    with tile.TileContext(nc) as tc, ExitStack() as ctx:
        singles = ctx.enter_context(tc.tile_pool(name="singles", bufs=1))
        trash_pool = ctx.enter_context(tc.tile_pool(name="tra", bufs=1))
        smalls = ctx.enter_context(tc.tile_pool(name="sm", bufs=2))
        xh_pool = ctx.enter_context(tc.tile_pool(name="xhp", bufs=2))
        dbls = ctx.enter_context(tc.tile_pool(name="dbl", bufs=3))
        dram = ctx.enter_context(tc.tile_pool(name="dram", bufs=1, space="DRAM"))
        psum_g = ctx.enter_context(tc.tile_pool(name="psg", bufs=1, space="PSUM"))
        psum_tr = ctx.enter_context(tc.tile_pool(name="pst", bufs=2, space="PSUM"))
        psum_o = ctx.enter_context(tc.tile_pool(name="pso", bufs=2, space="PSUM"))

        # double-buffered input; ones column written once per buffer, the
        # input DMAs only touch [:, :, 0:D]
        NBUF = 2
        itp = ctx.enter_context(tc.tile_pool(name="itp", bufs=1))
        it_bufs = [itp.tile([P, OWN_T, DA], F32, name=f"inbuf{b}")
                   for b in range(NBUF)]
        for b in range(NBUF):
            nc.gpsimd.memset(it_bufs[b][:, :, D], 1.0)
        ident = singles.tile([P, P], F32)
        make_identity(nc, ident)

        bis = [dram.tile([P, 2, DA], F32, name=f"cc_in{b}") for b in range(2)]
        bos = [dram.tile([P, 2, DA], F32, name=f"cc_out{b}") for b in range(2)]

        def pre_cc(i):
            """input load, norms, partial Gram, AllReduce launch, transposes.
            Returns state phase 2 needs."""
            it = it_bufs[i % NBUF]
            bi, bo = bis[i % 2], bos[i % 2]
            # 1 MB load in two halves (same ring) so the ScalarE norm path
            # starts after the first 512 KB lands
            nc.sync.dma_start(out=it[:, 0:OWN_T // 2, 0:D],
                              in_=xo[:, 0:OWN_T // 2, :])
            nc.sync.dma_start(out=it[:, OWN_T // 2:, 0:D],
                              in_=xo[:, OWN_T // 2:, :])

            # all-ScalarE norm path: 8x Square+accum, one batched sqrt.
            # dedicated trash slot per op: a reused slot would add a WAW
            # semaphore and Activation allows only one wait
            nsq = smalls.tile([P, OWN_T], F32, tag="nsq")
            for j in range(OWN_T):
                tr = trash_pool.tile([P, D], F32, tag=f"ta{j}")
                nc.scalar.activation(
                    out=tr, in_=it[:, j, 0:D], func=AF.Square,
                    accum_out=nsq[:, j:j + 1],
                )
            n0 = smalls.tile([P, OWN_T], F32, tag="n0")
            nc.scalar.activation(out=n0, in_=nsq, func=AF.Sqrt)
            r = smalls.tile([P, OWN_T], F32, tag="r")
            nc.vector.reciprocal(r, n0)

            # one batched scale on DVE: xh = x * (1/norm)
            xh = xh_pool.tile([P, OWN_T, D], F32, name="xh", tag="xh")
            rb = r.unsqueeze(2)
            nc.vector.tensor_mul(
                xh, it[:, :, 0:D], rb.to_broadcast([P, OWN_T, D]))

            # partial Gram G'_i = xhat_own.T @ [x_own | 1]:
            # single 2-bank PSUM tile so ONE copy drains both halves
            g_big = psum_g.tile([P, 2, 512], F32, name="g_big", tag="g")
            for j in range(OWN_T):
                for m in range(2):
                    nc.tensor.matmul(
                        g_big[:, m, 0:DA], lhsT=xh[:, j, m * P:(m + 1) * P],
                        rhs=it[:, j, :],
                        start=(j == 0), stop=(j == OWN_T - 1),
                    )

            # 8-core AllReduce of the 263 KB partial Gram
            gpart = dbls.tile([P, 2, DA], F32, name="gpart", tag="gpart")
            nc.scalar.copy(out=gpart, in_=g_big[:, :, 0:DA])
            nc.gpsimd.dma_start(bi[:, :, :], gpart[:, :, :])
            nc.gpsimd.collective_compute(
                "AllReduce",
                mybir.AluOpType.add,
                replica_groups=[list(range(NCORES))],
                ins=[bi.opt()],
                outs=[bo.opt()],
            )
            gsb = dbls.tile([P, 2, DA], F32, name="gsb", tag="gsb")
            nc.gpsimd.dma_start(gsb[:, :, :], bo[:, :, :])

            # own-block transposes (PE busy during the AllReduce)
            xT = [dbls.tile([P, OWN], F32, name=f"xT{dt}", tag=f"xT{dt}")
                  for dt in range(2)]
            for dt in range(2):
                for g in range(2):          # 4 transposes per PSUM bank
                    pst = psum_tr.tile([P, 4 * P], F32, name="pst", tag="tr")
                    for jj in range(4):
                        j = g * 4 + jj
                        nc.tensor.transpose(
                            pst[:, jj * P:(jj + 1) * P],
                            it[:, j, dt * P:(dt + 1) * P], ident,
                        )
                    nc.scalar.copy(out=xT[dt][:, g * 4 * P:(g + 1) * 4 * P],
                                   in_=pst)
            return gsb, xT

        def phase2(gsb, xT):
            """own rows x G'; batched row-sum reciprocal + divide; store."""
            outsb = dbls.tile([P, OWN_T, DA], F32, name="outsb", tag="outsb")
            for g2 in range(OWN_T // 2):
                oa = psum_o.tile([P, 2, 512], F32, name="oa", tag="oa")
                for jj in range(2):
                    j = g2 * 2 + jj
                    for k in range(2):
                        nc.tensor.matmul(
                            oa[:, jj, 0:DA], lhsT=xT[k][:, j * P:(j + 1) * P],
                            rhs=gsb[:, k, :],
                            start=(k == 0), stop=(k == 1),
                        )
                if g2 % 2 == 0:
                    nc.scalar.copy(out=outsb[:, g2 * 2:g2 * 2 + 2, :],
                                   in_=oa[:, :, 0:DA])
                else:
                    nc.vector.tensor_copy(out=outsb[:, g2 * 2:g2 * 2 + 2, :],
                                          in_=oa[:, :, 0:DA])
            rcp8 = smalls.tile([P, OWN_T], F32, tag="rcp8")
            nc.vector.reciprocal(rcp8, outsb[:, :, D])
            rcb = rcp8.unsqueeze(2)
            outfin = dbls.tile([P, OWN_T, D], F32, name="outfin", tag="outfin")
            H = OWN_T // 2
            nc.vector.tensor_mul(
                outfin[:, 0:H, :], outsb[:, 0:H, 0:D],
                rcb[:, 0:H, :].to_broadcast([P, H, D]))
            nc.sync.dma_start(out=ov[:, 0:H, :], in_=outfin[:, 0:H, :])
            nc.vector.tensor_mul(
                outfin[:, H:, :], outsb[:, H:, 0:D],
                rcb[:, H:, :].to_broadcast([P, OWN_T - H, D]))
            nc.sync.dma_start(out=ov[:, H:, :], in_=outfin[:, H:, :])

        # software pipeline, depth 2: phase 2 of iter i is emitted after the
        # cc launch of iter i+2, so by emission order every wait is already
        # satisfied — no engine queue ever blocks on an in-flight AllReduce
        pending = []
        for _it in range(iters):
            pending.append(pre_cc(_it))
            if len(pending) > 2:
                phase2(*pending.pop(0))
        while pending:
            phase2(*pending.pop(0))
    return nc


def _get_nc(iters: int = 1):
    if iters not in _nc_cache:
        nc = _build_nc(iters)
        orig = nc.to_json_bytes
        nc.to_json_bytes = lambda: _legalize_sync_waits(orig())
        _nc_cache[iters] = nc
    return _nc_cache[iters]


LAST_RESULTS = None  # BassKernelResults of the most recent run (for profiling)


def kernel(tensor: np.ndarray, trace: bool = False, **trace_kwargs) -> np.ndarray:
    x = np.ascontiguousarray(np.asarray(tensor, dtype=np.float32))
    assert x.shape == (N, D)
    nc = _get_nc()
    in_maps = [
        {"x_own": np.ascontiguousarray(x[i * OWN:(i + 1) * OWN])}
        for i in range(NCORES)
    ]
    global LAST_RESULTS
    LAST_RESULTS = run_bass_kernel_spmd(
        nc, in_maps, core_ids=list(range(NCORES)), trace=trace, **trace_kwargs
    )
    return np.concatenate([r["out"] for r in LAST_RESULTS.results], axis=0)


# revision 5
# speedup vs baseline: 1.0261x; 1.0261x over previous
"""Trainium2 Bass kernel: cosine-similarity message passing (GNN aggregate).

Math (collapsed — the [N,N] similarity matrix is never materialized):
    x_hat = x / max(||x||, eps)                      row-normalized features
    G'    = x_hat.T @ [x | 1]        [D, D+1]        Gram + column-sum s
    oa    = x @ G'                   [N, D+1]        (query-side normalization
                                                      cancels in the ratio)
    out   = oa[:, :D] / oa[:, D:D+1]
Precision: everything fp32. fp32r (=TF32) matmuls were measured SLOWER
in situ (extra rounding passes outweigh the 2x PE matmul speedup), and
any reduced precision on the s/row_sum path explodes the error on
near-cancelling rows (|row_sum| ~ 1e-3 rows dominate the fro norm).

Sharding: each core loads ONLY its own N/8 x D row block (1 MB),
computes the partial Gram over its rows, and an 8-core AllReduce of the
263 KB G' produces the full Gram everywhere. The x_own transposes for
phase 2 run on PE during the AllReduce wait.

v8 (this version): ~10% faster than the original baseline in
interleaved A/B (25.3us vs 28.0us steady-state per-iter; the
environment drifts +-25% between minutes, so ONLY interleaved
comparisons are trustworthy). Evidence says the kernel is close to
HBM-bandwidth-bound: the 8 cores share chip HBM at ~160 GB/s/core
marginal (adding a 1MB/iter input measured +6.2us), so in 1MB +
out 1MB + cc bounces ~1MB set a ~19us floor; compute-only (no DMA/cc)
runs at ~17us. Changes vs baseline:
 - norms: all 8 row-tiles via ScalarE Square+accum (drops the DVE
   bn_stats/bn_aggr path: -10 DVE ops), one batched Sqrt.
 - scale x_hat = x * (1/norm): ONE batched DVE multiply.
 - phase 2: matmuls into 2-bank PSUM groups, then multiply DIRECTLY
   out of PSUM (strided 2-wide reciprocal of the row-sums + broadcast
   multiply per group) — no oa staging copies, no outsb.
 - transpose drain copies alternate ScalarE/DVE.
 - AllReduce in-situ marginal cost is only ~3.4us in the pipelined
   steady state (latency ~13us is hidden by the depth-2 software
   pipeline).
Measured dead ends: fp32r(=TF32) matmuls anywhere (+1.2us: rounding
passes cost more than PE saves; also needs even free dims); host-
pretransposed x^T input to kill the 16 PE transposes (+6.2us: the
extra 1MB/iter DMA loses to HBM bandwidth).

The iters>1 builds (used by the steady-state delta timing) are
software-pipelined depth 2: phase 2 of iter i is emitted after the cc
launch of iter i+2, so no engine queue ever blocks on an in-flight
AllReduce.

Environment quirks:
 - this walrus build accepts at most ONE sync wait per instruction:
   _legalize_sync_waits hoists extras onto same-engine Drain carriers.
 - Rsqrt/Reciprocal activation funcs are disabled in bass (accuracy);
   norm recip = ACT Sqrt + DVE reciprocal.
 - eps in max(||x||, eps) never binds for gaussian rows (min norm ~14).
 - DRAM-bounce collectives (SBUF collectives are disabled in bass), all
   cc-adjacent DMAs on the gpsimd queue for straight-line ordering.
"""

import numpy as np
from contextlib import ExitStack

import concourse.bass as bass
import concourse.tile as tile
from concourse import mybir
from concourse.masks import make_identity
from concourse.bass_utils import run_bass_kernel_spmd

N, D = 8192, 256
NCORES = 8
P = 128
OWN = N // NCORES            # 1024 rows per core
OWN_T = OWN // P             # 8 own tiles
DA = D + 1                   # 257: x columns + ones column
F32 = mybir.dt.float32
AF = mybir.ActivationFunctionType

_nc_cache = {}


def _legalize_sync_waits(bir_bytes: bytes) -> bytes:
    """This walrus build accepts at most ONE sync wait per instruction.
    Tile emits several; hoist the extras onto same-engine Drain
    instructions placed immediately before (queue order preserves the
    semantics of inline waits)."""
    import orjson
    bir = orjson.loads(bir_bytes)
    ctr = [0]

    def fix_block(blk):
        new_list = []
        for ins in blk.get("instructions", []):
            si = ins.get("sync_info")
            if si:
                waits = si.get("on_wait") or []
                if len(waits) > 1:
                    for w in waits[:-1]:
                        ctr[0] += 1
                        new_list.append({
                            "debug": ins.get("debug", 0),
                            "engine": ins["engine"],
                            "ins": [], "outs": [],
                            "name": f"I-lw{ctr[0]}",
                            "opcode": "Drain",
                            "sync_info": {"on_update": [], "on_wait": [w]},
                        })
                    si["on_wait"] = waits[-1:]
            new_list.append(ins)
        blk["instructions"] = new_list
        for sb in blk.get("blocks", []):
            fix_block(sb)

    for f in bir["functions"]:
        for blk in f["blocks"]:
            fix_block(blk)
    return orjson.dumps(bir)


def _build_nc(iters: int = 1):
    nc = bass.Bass(
        "TRN2", target_bir_lowering=False, debug=False, enable_asserts=True,
        num_devices=NCORES,
    )
    x_own = nc.declare_dram_parameter("x_own", [OWN, D], F32, isOutput=False)
    out = nc.declare_dram_parameter("out", [OWN, D], F32, isOutput=True)

    # row order: row = p*8 + t  -> 8 KB contiguous HBM reads per partition
    xo = x_own.ap().rearrange("(p t) d -> p t d", p=P)
    ov = out.ap().rearrange("(p t) d -> p t d", p=P)

    with tile.TileContext(nc) as tc, ExitStack() as ctx:
        singles = ctx.enter_context(tc.tile_pool(name="singles", bufs=1))
        trash_pool = ctx.enter_context(tc.tile_pool(name="tra", bufs=1))
        smalls = ctx.enter_context(tc.tile_pool(name="sm", bufs=2))
        xh_pool = ctx.enter_context(tc.tile_pool(name="xhp", bufs=2))
        dbls = ctx.enter_context(tc.tile_pool(name="dbl", bufs=3))
        dram = ctx.enter_context(tc.tile_pool(name="dram", bufs=1, space="DRAM"))
        psum_g = ctx.enter_context(tc.tile_pool(name="psg", bufs=1, space="PSUM"))
        psum_tr = ctx.enter_context(tc.tile_pool(name="pst", bufs=2, space="PSUM"))
        psum_o = ctx.enter_context(tc.tile_pool(name="pso", bufs=2, space="PSUM"))

        # double-buffered input; ones column written once per buffer, the
        # input DMAs only touch [:, :, 0:D]
        NBUF = 2
        itp = ctx.enter_context(tc.tile_pool(name="itp", bufs=1))
        it_bufs = [itp.tile([P, OWN_T, DA], F32, name=f"inbuf{b}")
                   for b in range(NBUF)]
        for b in range(NBUF):
            nc.gpsimd.memset(it_bufs[b][:, :, D], 1.0)
        ident = singles.tile([P, P], F32)
        make_identity(nc, ident)

        bis = [dram.tile([P, 2, DA], F32, name=f"cc_in{b}") for b in range(2)]
        bos = [dram.tile([P, 2, DA], F32, name=f"cc_out{b}") for b in range(2)]

        def pre_cc(i):
            """input load, norms, partial Gram, AllReduce launch, transposes.
            Returns state phase 2 needs."""
            it = it_bufs[i % NBUF]
            bi, bo = bis[i % 2], bos[i % 2]
            # 1 MB load in two halves (same ring) so the ScalarE norm path
            # starts after the first 512 KB lands
            nc.sync.dma_start(out=it[:, 0:OWN_T // 2, 0:D],
                              in_=xo[:, 0:OWN_T // 2, :])
            nc.sync.dma_start(out=it[:, OWN_T // 2:, 0:D],
                              in_=xo[:, OWN_T // 2:, :])

            # all-ScalarE norm path: 8x Square+accum, one batched sqrt.
            # dedicated trash slot per op: a reused slot would add a WAW
            # semaphore and Activation allows only one wait
            nsq = smalls.tile([P, OWN_T], F32, tag="nsq")
            for j in range(OWN_T):
                tr = trash_pool.tile([P, D], F32, tag=f"ta{j}")
                nc.scalar.activation(
                    out=tr, in_=it[:, j, 0:D], func=AF.Square,
                    accum_out=nsq[:, j:j + 1],
                )
            n0 = smalls.tile([P, OWN_T], F32, tag="n0")
            nc.scalar.activation(out=n0, in_=nsq, func=AF.Sqrt)
            r = smalls.tile([P, OWN_T], F32, tag="r")
            nc.vector.reciprocal(r, n0)

            # one batched scale on DVE: xh = x * (1/norm)
            xh = xh_pool.tile([P, OWN_T, D], F32, name="xh", tag="xh")
            rb = r.unsqueeze(2)
            nc.vector.tensor_mul(
                xh, it[:, :, 0:D], rb.to_broadcast([P, OWN_T, D]))

            # partial Gram G'_i = xhat_own.T @ [x_own | 1]:
            # single 2-bank PSUM tile so ONE copy drains both halves
            g_big = psum_g.tile([P, 2, 512], F32, name="g_big", tag="g")
            for j in range(OWN_T):
                for m in range(2):
                    nc.tensor.matmul(
                        g_big[:, m, 0:DA], lhsT=xh[:, j, m * P:(m + 1) * P],
                        rhs=it[:, j, :],
                        start=(j == 0), stop=(j == OWN_T - 1),
                    )

            # 8-core AllReduce of the 263 KB partial Gram
            gpart = dbls.tile([P, 2, DA], F32, name="gpart", tag="gpart")
            nc.scalar.copy(out=gpart, in_=g_big[:, :, 0:DA])
            nc.gpsimd.dma_start(bi[:, :, :], gpart[:, :, :])
            nc.gpsimd.collective_compute(
                "AllReduce",
                mybir.AluOpType.add,
                replica_groups=[list(range(NCORES))],
                ins=[bi.opt()],
                outs=[bo.opt()],
            )
            gsb = dbls.tile([P, 2, DA], F32, name="gsb", tag="gsb")
            nc.gpsimd.dma_start(gsb[:, :, :], bo[:, :, :])

            # own-block transposes (PE busy during the AllReduce)
            xT = [dbls.tile([P, OWN], F32, name=f"xT{dt}", tag=f"xT{dt}")
                  for dt in range(2)]
            for dt in range(2):
                for g in range(2):          # 4 transposes per PSUM bank
                    pst = psum_tr.tile([P, 4 * P], F32, name="pst", tag="tr")
                    for jj in range(4):
                        j = g * 4 + jj
                        nc.tensor.transpose(
                            pst[:, jj * P:(jj + 1) * P],
                            it[:, j, dt * P:(dt + 1) * P], ident,
                        )
                    if (dt + g) % 2 == 0:
                        nc.scalar.copy(
                            out=xT[dt][:, g * 4 * P:(g + 1) * 4 * P], in_=pst)
                    else:
                        nc.vector.tensor_copy(
                            out=xT[dt][:, g * 4 * P:(g + 1) * 4 * P], in_=pst)
            return gsb, xT

        def phase2(gsb, xT):
            """own rows x G'; per-group: multiply directly out of PSUM with
            a strided 2-wide reciprocal of the row-sums (no staging copy)."""
            outfin = dbls.tile([P, OWN_T, D], F32, name="outfin", tag="outfin")
            H = OWN_T // 2
            for g2 in range(OWN_T // 2):
                oa = psum_o.tile([P, 2, 512], F32, name="oa", tag="oa")
                for jj in range(2):
                    j = g2 * 2 + jj
                    for k in range(2):
                        nc.tensor.matmul(
                            oa[:, jj, 0:DA], lhsT=xT[k][:, j * P:(j + 1) * P],
                            rhs=gsb[:, k, :],
                            start=(k == 0), stop=(k == 1),
                        )
                rcp2 = smalls.tile([P, 2], F32, tag="rcp2")
                nc.vector.reciprocal(rcp2, oa[:, :, D])
                nc.vector.tensor_mul(
                    outfin[:, g2 * 2:g2 * 2 + 2, :], oa[:, :, 0:D],
                    rcp2.unsqueeze(2).to_broadcast([P, 2, D]))
                if g2 == OWN_T // 4 - 1:
                    nc.sync.dma_start(out=ov[:, 0:H, :],
                                      in_=outfin[:, 0:H, :])
            nc.sync.dma_start(out=ov[:, H:, :], in_=outfin[:, H:, :])

        # software pipeline, depth 2: phase 2 of iter i is emitted after the
        # cc launch of iter i+2, so by emission order every wait is already
        # satisfied — no engine queue ever blocks on an in-flight AllReduce
        pending = []
        for _it in range(iters):
            pending.append(pre_cc(_it))
            if len(pending) > 2:
                phase2(*pending.pop(0))
        while pending:
            phase2(*pending.pop(0))
    return nc


def _get_nc(iters: int = 1):
    if iters not in _nc_cache:
        nc = _build_nc(iters)
        orig = nc.to_json_bytes
        nc.to_json_bytes = lambda: _legalize_sync_waits(orig())
        _nc_cache[iters] = nc
    return _nc_cache[iters]


LAST_RESULTS = None  # BassKernelResults of the most recent run (for profiling)


def kernel(tensor: np.ndarray, trace: bool = False, **trace_kwargs) -> np.ndarray:
    x = np.ascontiguousarray(np.asarray(tensor, dtype=np.float32))
    assert x.shape == (N, D)
    nc = _get_nc()
    in_maps = [
        {"x_own": np.ascontiguousarray(x[i * OWN:(i + 1) * OWN])}
        for i in range(NCORES)
    ]
    global LAST_RESULTS
    LAST_RESULTS = run_bass_kernel_spmd(
        nc, in_maps, core_ids=list(range(NCORES)), trace=trace, **trace_kwargs
    )
    return np.concatenate([r["out"] for r in LAST_RESULTS.results], axis=0)


# revision 9
# speedup vs baseline: 1.1101x; 1.0818x over previous
"""Trainium2 Bass kernel: cosine-similarity message passing (GNN aggregate).

Math (collapsed — the [N,N] similarity matrix is never materialized):
    x_hat = x / max(||x||, eps)                      row-normalized features
    G'    = x_hat.T @ [x | 1]        [D, D+1]        Gram + column-sum s
    oa    = x @ G'                   [N, D+1]        (query-side normalization
                                                      cancels in the ratio)
    out   = oa[:, :D] / oa[:, D:D+1]
Precision: everything fp32. fp32r (=TF32) matmuls were measured SLOWER
in situ (extra rounding passes outweigh the 2x PE matmul speedup), and
any reduced precision on the s/row_sum path explodes the error on
near-cancelling rows (|row_sum| ~ 1e-3 rows dominate the fro norm).

Sharding: each core loads ONLY its own N/8 x D row block (1 MB),
computes the partial Gram over its rows, and an 8-core AllReduce of the
263 KB G' produces the full Gram everywhere. The x_own transposes for
phase 2 run on PE during the AllReduce wait.

v14 (this version): ~13% faster than the original baseline in
interleaved A/B (24.4us vs ~28us steady-state per-iter; the
environment drifts +-25% between minutes, so ONLY interleaved
comparisons are trustworthy). Floor decomposition (measured by probe
variants): the PE stream alone (16 Gram + 16 transpose + 16 phase2
fp32 matmuls at 380ns) runs at ~17.2us with its PSUM drains; input
DMA adds ~2.7us/MB (~370 GB/s marginal), output similar, AllReduce
in-situ marginal ~3.4us (its ~13us latency is hidden by the depth-2
software pipeline). Changes vs baseline:
 - norms: all 8 row-tiles via ScalarE Square+accum (drops the DVE
   bn_stats/bn_aggr path), one batched Sqrt. ScalarE does ONLY ACT
   work; restoring the split ScalarE/DVE norm path measured SLOWER.
 - scale x_hat = x * (1/norm): ONE batched multiply on Pool (GpSimd
   is otherwise idle).
 - all PSUM drains (Gram copy, 4 transpose-drain copies) on DVE.
 - phase 2: matmuls into 2-bank PSUM groups, then multiply DIRECTLY
   out of PSUM (strided 2-wide reciprocal of the row-sums + broadcast
   multiply per group) — no oa staging copies, no outsb.
Measured dead ends (do not retry): fp32r(=TF32) matmuls anywhere
(+1.2us: rounding-producer passes cost more than PE saves; even free
dims required); host-pretransposed x^T input to kill the 16 PE
transposes (+6.2us: the extra 1MB/iter DMA loses); cc payload shrink
514->386 words via symmetric-block packing (+11us: the collective
hates it); output DMAs on the scalar queue (+2.6us); NBUF=3 + split
front-end (+2.7us).

The iters>1 builds (used by the steady-state delta timing) are
software-pipelined depth 2: phase 2 of iter i is emitted after the cc
launch of iter i+2, so no engine queue ever blocks on an in-flight
AllReduce.

Environment quirks:
 - this walrus build accepts at most ONE sync wait per instruction:
   _legalize_sync_waits hoists extras onto same-engine Drain carriers.
 - Rsqrt/Reciprocal activation funcs are disabled in bass (accuracy);
   norm recip = ACT Sqrt + DVE reciprocal.
 - eps in max(||x||, eps) never binds for gaussian rows (min norm ~14).
 - DRAM-bounce collectives (SBUF collectives are disabled in bass), all
   cc-adjacent DMAs on the gpsimd queue for straight-line ordering.
"""

import numpy as np
from contextlib import ExitStack

import concourse.bass as bass
import concourse.tile as tile
from concourse import mybir
from concourse.masks import make_identity
from concourse.bass_utils import run_bass_kernel_spmd

N, D = 8192, 256
NCORES = 8
P = 128
OWN = N // NCORES            # 1024 rows per core
OWN_T = OWN // P             # 8 own tiles
DA = D + 1                   # 257: x columns + ones column
F32 = mybir.dt.float32
AF = mybir.ActivationFunctionType

_nc_cache = {}


def _legalize_sync_waits(bir_bytes: bytes) -> bytes:
    """This walrus build accepts at most ONE sync wait per instruction.
    Tile emits several; hoist the extras onto same-engine Drain
    instructions placed immediately before (queue order preserves the
    semantics of inline waits)."""
    import orjson
    bir = orjson.loads(bir_bytes)
    ctr = [0]

    def fix_block(blk):
        new_list = []
        for ins in blk.get("instructions", []):
            si = ins.get("sync_info")
            if si:
                waits = si.get("on_wait") or []
                if len(waits) > 1:
                    for w in waits[:-1]:
                        ctr[0] += 1
                        new_list.append({
                            "debug": ins.get("debug", 0),
                            "engine": ins["engine"],
                            "ins": [], "outs": [],
                            "name": f"I-lw{ctr[0]}",
                            "opcode": "Drain",
                            "sync_info": {"on_update": [], "on_wait": [w]},
                        })
                    si["on_wait"] = waits[-1:]
            new_list.append(ins)
        blk["instructions"] = new_list
        for sb in blk.get("blocks", []):
            fix_block(sb)

    for f in bir["functions"]:
        for blk in f["blocks"]:
            fix_block(blk)
    return orjson.dumps(bir)


def _build_nc(iters: int = 1):
    nc = bass.Bass(
        "TRN2", target_bir_lowering=False, debug=False, enable_asserts=True,
        num_devices=NCORES,
    )
    x_own = nc.declare_dram_parameter("x_own", [OWN, D], F32, isOutput=False)
    out = nc.declare_dram_parameter("out", [OWN, D], F32, isOutput=True)

    # row order: row = p*8 + t  -> 8 KB contiguous HBM reads per partition
    xo = x_own.ap().rearrange("(p t) d -> p t d", p=P)
    ov = out.ap().rearrange("(p t) d -> p t d", p=P)

    with tile.TileContext(nc) as tc, ExitStack() as ctx:
        singles = ctx.enter_context(tc.tile_pool(name="singles", bufs=1))
        trash_pool = ctx.enter_context(tc.tile_pool(name="tra", bufs=1))
        smalls = ctx.enter_context(tc.tile_pool(name="sm", bufs=2))
        xh_pool = ctx.enter_context(tc.tile_pool(name="xhp", bufs=2))
        dbls = ctx.enter_context(tc.tile_pool(name="dbl", bufs=3))
        dram = ctx.enter_context(tc.tile_pool(name="dram", bufs=1, space="DRAM"))
        psum_g = ctx.enter_context(tc.tile_pool(name="psg", bufs=1, space="PSUM"))
        psum_tr = ctx.enter_context(tc.tile_pool(name="pst", bufs=2, space="PSUM"))
        psum_o = ctx.enter_context(tc.tile_pool(name="pso", bufs=2, space="PSUM"))

        # double-buffered input; ones column written once per buffer, the
        # input DMAs only touch [:, :, 0:D]
        NBUF = 2
        itp = ctx.enter_context(tc.tile_pool(name="itp", bufs=1))
        it_bufs = [itp.tile([P, OWN_T, DA], F32, name=f"inbuf{b}")
                   for b in range(NBUF)]
        for b in range(NBUF):
            nc.gpsimd.memset(it_bufs[b][:, :, D], 1.0)
        ident = singles.tile([P, P], F32)
        make_identity(nc, ident)

        bis = [dram.tile([P, 2, DA], F32, name=f"cc_in{b}") for b in range(2)]
        bos = [dram.tile([P, 2, DA], F32, name=f"cc_out{b}") for b in range(2)]

        def pre_cc(i):
            """input load, norms, partial Gram, AllReduce launch, transposes.
            Returns state phase 2 needs."""
            it = it_bufs[i % NBUF]
            bi, bo = bis[i % 2], bos[i % 2]
            # 1 MB load in two halves (same ring) so the ScalarE norm path
            # starts after the first 512 KB lands
            nc.sync.dma_start(out=it[:, 0:OWN_T // 2, 0:D],
                              in_=xo[:, 0:OWN_T // 2, :])
            nc.sync.dma_start(out=it[:, OWN_T // 2:, 0:D],
                              in_=xo[:, OWN_T // 2:, :])

            # all-ScalarE norm path: 8x Square+accum, one batched sqrt.
            # dedicated trash slot per op: a reused slot would add a WAW
            # semaphore and Activation allows only one wait
            nsq = smalls.tile([P, OWN_T], F32, tag="nsq")
            for j in range(OWN_T):
                tr = trash_pool.tile([P, D], F32, tag=f"ta{j}")
                nc.scalar.activation(
                    out=tr, in_=it[:, j, 0:D], func=AF.Square,
                    accum_out=nsq[:, j:j + 1],
                )
            n0 = smalls.tile([P, OWN_T], F32, tag="n0")
            nc.scalar.activation(out=n0, in_=nsq, func=AF.Sqrt)
            r = smalls.tile([P, OWN_T], F32, tag="r")
            nc.vector.reciprocal(r, n0)

            # one batched scale xh = x * (1/norm) on Pool (otherwise idle)
            xh = xh_pool.tile([P, OWN_T, D], F32, name="xh", tag="xh")
            rb = r.unsqueeze(2)
            nc.gpsimd.tensor_mul(
                xh, it[:, :, 0:D], rb.to_broadcast([P, OWN_T, D]))

            # partial Gram G'_i = xhat_own.T @ [x_own | 1]:
            # single 2-bank PSUM tile so ONE copy drains both halves
            g_big = psum_g.tile([P, 2, 512], F32, name="g_big", tag="g")
            for j in range(OWN_T):
                for m in range(2):
                    nc.tensor.matmul(
                        g_big[:, m, 0:DA], lhsT=xh[:, j, m * P:(m + 1) * P],
                        rhs=it[:, j, :],
                        start=(j == 0), stop=(j == OWN_T - 1),
                    )

            # 8-core AllReduce of the 263 KB partial Gram
            gpart = dbls.tile([P, 2, DA], F32, name="gpart", tag="gpart")
            nc.vector.tensor_copy(out=gpart, in_=g_big[:, :, 0:DA])
            nc.gpsimd.dma_start(bi[:, :, :], gpart[:, :, :])
            nc.gpsimd.collective_compute(
                "AllReduce",
                mybir.AluOpType.add,
                replica_groups=[list(range(NCORES))],
                ins=[bi.opt()],
                outs=[bo.opt()],
            )
            gsb = dbls.tile([P, 2, DA], F32, name="gsb", tag="gsb")
            nc.gpsimd.dma_start(gsb[:, :, :], bo[:, :, :])

            # own-block transposes (PE busy during the AllReduce)
            xT = [dbls.tile([P, OWN], F32, name=f"xT{dt}", tag=f"xT{dt}")
                  for dt in range(2)]
            for dt in range(2):
                for g in range(2):          # 4 transposes per PSUM bank
                    pst = psum_tr.tile([P, 4 * P], F32, name="pst", tag="tr")
                    for jj in range(4):
                        j = g * 4 + jj
                        nc.tensor.transpose(
                            pst[:, jj * P:(jj + 1) * P],
                            it[:, j, dt * P:(dt + 1) * P], ident,
                        )
                    nc.vector.tensor_copy(
                        out=xT[dt][:, g * 4 * P:(g + 1) * 4 * P], in_=pst)
            return gsb, xT

        def phase2(gsb, xT):
            """own rows x G'; per-group: multiply directly out of PSUM with
            a strided 2-wide reciprocal of the row-sums (no staging copy)."""
            outfin = dbls.tile([P, OWN_T, D], F32, name="outfin", tag="outfin")
            H = OWN_T // 2
            for g2 in range(OWN_T // 2):
                oa = psum_o.tile([P, 2, 512], F32, name="oa", tag="oa")
                for jj in range(2):
                    j = g2 * 2 + jj
                    for k in range(2):
                        nc.tensor.matmul(
                            oa[:, jj, 0:DA], lhsT=xT[k][:, j * P:(j + 1) * P],
                            rhs=gsb[:, k, :],
                            start=(k == 0), stop=(k == 1),
                        )
                rcp2 = smalls.tile([P, 2], F32, tag="rcp2")
                nc.vector.reciprocal(rcp2, oa[:, :, D])
                nc.vector.tensor_mul(
                    outfin[:, g2 * 2:g2 * 2 + 2, :], oa[:, :, 0:D],
                    rcp2.unsqueeze(2).to_broadcast([P, 2, D]))
                if g2 == OWN_T // 4 - 1:
                    nc.sync.dma_start(out=ov[:, 0:H, :],
                                      in_=outfin[:, 0:H, :])
            nc.sync.dma_start(out=ov[:, H:, :], in_=outfin[:, H:, :])

        # software pipeline, depth 2: phase 2 of iter i is emitted after the
        # cc launch of iter i+2, so by emission order every wait is already
        # satisfied — no engine queue ever blocks on an in-flight AllReduce
        pending = []
        for _it in range(iters):
            pending.append(pre_cc(_it))
            if len(pending) > 2:
                phase2(*pending.pop(0))
        while pending:
            phase2(*pending.pop(0))
    return nc


def _get_nc(iters: int = 1):
    if iters not in _nc_cache:
        nc = _build_nc(iters)
        orig = nc.to_json_bytes
        nc.to_json_bytes = lambda: _legalize_sync_waits(orig())
        _nc_cache[iters] = nc
    return _nc_cache[iters]


LAST_RESULTS = None  # BassKernelResults of the most recent run (for profiling)


def kernel(tensor: np.ndarray, trace: bool = False, **trace_kwargs) -> np.ndarray:
    x = np.ascontiguousarray(np.asarray(tensor, dtype=np.float32))
    assert x.shape == (N, D)
    nc = _get_nc()
    in_maps = [
        {"x_own": np.ascontiguousarray(x[i * OWN:(i + 1) * OWN])}
        for i in range(NCORES)
    ]
    global LAST_RESULTS
    LAST_RESULTS = run_bass_kernel_spmd(
        nc, in_maps, core_ids=list(range(NCORES)), trace=trace, **trace_kwargs
    )
    return np.concatenate([r["out"] for r in LAST_RESULTS.results], axis=0)


# revision 11
# speedup vs baseline: 1.2096x; 1.0897x over previous
"""Trainium2 Bass kernel: cosine-similarity message passing (GNN aggregate).

Math (collapsed — the [N,N] similarity matrix is never materialized):
    x_hat = x / max(||x||, eps)                      row-normalized features
    G'    = x_hat.T @ [x | 1]        [D, D+1]        Gram + column-sum s
    oa    = x @ G'                   [N, D+1]        (query-side normalization
                                                      cancels in the ratio)
    out   = oa[:, :D] / oa[:, D:D+1]
Precision: everything fp32. fp32r (=TF32) matmuls were measured SLOWER
in situ (extra rounding passes outweigh the 2x PE matmul speedup), and
any reduced precision on the s/row_sum path explodes the error on
near-cancelling rows (|row_sum| ~ 1e-3 rows dominate the fro norm).

Sharding: each core loads ONLY its own N/8 x D row block (1 MB),
computes the partial Gram over its rows, and an 8-core AllReduce of the
263 KB G' produces the full Gram everywhere. The x_own transposes for
phase 2 run on PE during the AllReduce wait.

v20 (this version): ~21% faster than the original baseline in
interleaved A/B (24.4us vs ~28us steady-state per-iter; the
environment drifts +-25% between minutes, so ONLY interleaved
comparisons are trustworthy). Floor decomposition (measured by probe
variants): the PE stream alone (16 Gram + 16 transpose + 16 phase2
fp32 matmuls at 380ns) runs at ~17.2us with its PSUM drains; input
DMA adds ~2.7us/MB (~370 GB/s marginal), output similar, AllReduce
in-situ marginal ~3.4us (its ~13us latency is hidden by the depth-2
software pipeline). Changes vs baseline:
 - norms: all 8 row-tiles via ScalarE Square+accum (drops the DVE
   bn_stats/bn_aggr path), one batched Sqrt. ScalarE does ONLY ACT
   work; restoring the split ScalarE/DVE norm path measured SLOWER.
 - scale x_hat = x * (1/norm): ONE batched multiply on Pool (GpSimd
   is otherwise idle).
 - all PSUM drains (Gram copy, 4 transpose-drain copies) on DVE.
 - phase 2: matmuls into 2-bank PSUM groups, then multiply DIRECTLY
   out of PSUM (strided 2-wide reciprocal of the row-sums + broadcast
   multiply per group) — no oa staging copies, no outsb.
Measured dead ends (do not retry): fp32r(=TF32) matmuls anywhere
(+1.2us: rounding-producer passes cost more than PE saves; even free
dims required); host-pretransposed x^T input to kill the 16 PE
transposes (+6.2us: the extra 1MB/iter DMA loses); cc payload shrink
514->386 words via symmetric-block packing (+11us: the collective
hates it); output DMAs on the scalar queue (+2.6us); NBUF=3 + split
front-end (+2.7us).

The iters>1 builds (used by the steady-state delta timing) are
software-pipelined depth 2: phase 2 of iter i is emitted after the cc
launch of iter i+2, so no engine queue ever blocks on an in-flight
AllReduce.

Environment quirks:
 - this walrus build accepts at most ONE sync wait per instruction:
   _legalize_sync_waits hoists extras onto same-engine Drain carriers.
 - Rsqrt/Reciprocal activation funcs are disabled in bass (accuracy);
   norm recip = ACT Sqrt + DVE reciprocal.
 - eps in max(||x||, eps) never binds for gaussian rows (min norm ~14).
 - DRAM-bounce collectives (SBUF collectives are disabled in bass), all
   cc-adjacent DMAs on the gpsimd queue for straight-line ordering.
"""

import numpy as np
from contextlib import ExitStack

import concourse.bass as bass
import concourse.tile as tile
from concourse import mybir
from concourse.masks import make_identity
from concourse.bass_utils import run_bass_kernel_spmd

N, D = 8192, 256
NCORES = 8
P = 128
OWN = N // NCORES            # 1024 rows per core
OWN_T = OWN // P             # 8 own tiles
DA = D + 1                   # 257: x columns + ones column
F32 = mybir.dt.float32
AF = mybir.ActivationFunctionType

_nc_cache = {}


def _legalize_sync_waits(bir_bytes: bytes) -> bytes:
    """This walrus build accepts at most ONE sync wait per instruction.
    Tile emits several; hoist the extras onto same-engine Drain
    instructions placed immediately before (queue order preserves the
    semantics of inline waits)."""
    import orjson
    bir = orjson.loads(bir_bytes)
    ctr = [0]

    def fix_block(blk):
        new_list = []
        for ins in blk.get("instructions", []):
            si = ins.get("sync_info")
            if si:
                waits = si.get("on_wait") or []
                if len(waits) > 1:
                    for w in waits[:-1]:
                        ctr[0] += 1
                        new_list.append({
                            "debug": ins.get("debug", 0),
                            "engine": ins["engine"],
                            "ins": [], "outs": [],
                            "name": f"I-lw{ctr[0]}",
                            "opcode": "Drain",
                            "sync_info": {"on_update": [], "on_wait": [w]},
                        })
                    si["on_wait"] = waits[-1:]
            new_list.append(ins)
        blk["instructions"] = new_list
        for sb in blk.get("blocks", []):
            fix_block(sb)

    for f in bir["functions"]:
        for blk in f["blocks"]:
            fix_block(blk)
    return orjson.dumps(bir)


def _build_nc(iters: int = 1):
    nc = bass.Bass(
        "TRN2", target_bir_lowering=False, debug=False, enable_asserts=True,
        num_devices=NCORES,
    )
    x_own = nc.declare_dram_parameter("x_own", [OWN, D], F32, isOutput=False)
    out = nc.declare_dram_parameter("out", [OWN, D], F32, isOutput=True)

    # row order: row = p*8 + t  -> 8 KB contiguous HBM reads per partition
    xo = x_own.ap().rearrange("(p t) d -> p t d", p=P)
    ov = out.ap().rearrange("(p t) d -> p t d", p=P)

    with tile.TileContext(nc) as tc, ExitStack() as ctx:
        singles = ctx.enter_context(tc.tile_pool(name="singles", bufs=1))
        trash_pool = ctx.enter_context(tc.tile_pool(name="tra", bufs=1))
        smalls = ctx.enter_context(tc.tile_pool(name="sm", bufs=2))
        xh_pool = ctx.enter_context(tc.tile_pool(name="xhp", bufs=2))
        dbls = ctx.enter_context(tc.tile_pool(name="dbl", bufs=3))
        dram = ctx.enter_context(tc.tile_pool(name="dram", bufs=1, space="DRAM"))
        psum_g = ctx.enter_context(tc.tile_pool(name="psg", bufs=1, space="PSUM"))
        psum_tr = ctx.enter_context(tc.tile_pool(name="pst", bufs=2, space="PSUM"))
        psum_o = ctx.enter_context(tc.tile_pool(name="pso", bufs=2, space="PSUM"))

        # double-buffered input; ones column written once per buffer, the
        # input DMAs only touch [:, :, 0:D]
        NBUF = 2
        itp = ctx.enter_context(tc.tile_pool(name="itp", bufs=1))
        it_bufs = [itp.tile([P, OWN_T, DA], F32, name=f"inbuf{b}")
                   for b in range(NBUF)]
        for b in range(NBUF):
            nc.gpsimd.memset(it_bufs[b][:, :, D], 1.0)
        ident = singles.tile([P, P], F32)
        make_identity(nc, ident)

        bis = [dram.tile([P, 2, DA], F32, name=f"cc_in{b}") for b in range(2)]
        # Shared-address-space collective output is the runtime's fast path
        # (-1.9us/iter measured). Shared DRAM allows only a single writing
        # instruction per tensor, so each iteration gets its own.
        bos = [dram.tile([P, 2, DA], F32, name=f"cc_out{b}",
                         addr_space="Shared", tag=f"bo{b}")
               for b in range(iters)]

        def pre_cc(i):
            """input load, norms, partial Gram, AllReduce launch, transposes.
            Returns state phase 2 needs."""
            it = it_bufs[i % NBUF]
            bi, bo = bis[i % 2], bos[i]
            # 1 MB load in two halves (same ring) so the ScalarE norm path
            # starts after the first 512 KB lands
            nc.sync.dma_start(out=it[:, 0:OWN_T // 2, 0:D],
                              in_=xo[:, 0:OWN_T // 2, :])
            nc.sync.dma_start(out=it[:, OWN_T // 2:, 0:D],
                              in_=xo[:, OWN_T // 2:, :])

            # all-ScalarE norm path: 8x Square+accum, one batched sqrt.
            # dedicated trash slot per op: a reused slot would add a WAW
            # semaphore and Activation allows only one wait
            nsq = smalls.tile([P, OWN_T], F32, tag="nsq")
            for j in range(OWN_T):
                tr = trash_pool.tile([P, D], F32, tag=f"ta{j}")
                nc.scalar.activation(
                    out=tr, in_=it[:, j, 0:D], func=AF.Square,
                    accum_out=nsq[:, j:j + 1],
                )
            n0 = smalls.tile([P, OWN_T], F32, tag="n0")
            nc.scalar.activation(out=n0, in_=nsq, func=AF.Sqrt)
            r = smalls.tile([P, OWN_T], F32, tag="r")
            nc.vector.reciprocal(r, n0)

            # one batched scale xh = x * (1/norm) on Pool (otherwise idle)
            xh = xh_pool.tile([P, OWN_T, D], F32, name="xh", tag="xh")
            rb = r.unsqueeze(2)
            nc.gpsimd.tensor_mul(
                xh, it[:, :, 0:D], rb.to_broadcast([P, OWN_T, D]))

            # partial Gram G'_i = xhat_own.T @ [x_own | 1]:
            # single 2-bank PSUM tile so ONE copy drains both halves
            g_big = psum_g.tile([P, 2, 512], F32, name="g_big", tag="g")
            for j in range(OWN_T):
                for m in range(2):
                    nc.tensor.matmul(
                        g_big[:, m, 0:DA], lhsT=xh[:, j, m * P:(m + 1) * P],
                        rhs=it[:, j, :],
                        start=(j == 0), stop=(j == OWN_T - 1),
                    )

            # 8-core AllReduce of the 263 KB partial Gram
            gpart = dbls.tile([P, 2, DA], F32, name="gpart", tag="gpart")
            nc.vector.tensor_copy(out=gpart, in_=g_big[:, :, 0:DA])
            nc.gpsimd.dma_start(bi[:, :, :], gpart[:, :, :])
            nc.gpsimd.collective_compute(
                "AllReduce",
                mybir.AluOpType.add,
                replica_groups=[list(range(NCORES))],
                ins=[bi.opt()],
                outs=[bo.opt()],
            )
            gsb = dbls.tile([P, 2, DA], F32, name="gsb", tag="gsb")
            nc.gpsimd.dma_start(gsb[:, :, :], bo[:, :, :])

            # own-block transposes (PE busy during the AllReduce)
            xT = [dbls.tile([P, OWN], F32, name=f"xT{dt}", tag=f"xT{dt}")
                  for dt in range(2)]
            for dt in range(2):
                for g in range(2):          # 4 transposes per PSUM bank
                    pst = psum_tr.tile([P, 4 * P], F32, name="pst", tag="tr")
                    for jj in range(4):
                        j = g * 4 + jj
                        nc.tensor.transpose(
                            pst[:, jj * P:(jj + 1) * P],
                            it[:, j, dt * P:(dt + 1) * P], ident,
                        )
                    nc.vector.tensor_copy(
                        out=xT[dt][:, g * 4 * P:(g + 1) * 4 * P], in_=pst)
            return gsb, xT

        def phase2(gsb, xT):
            """own rows x G'; per-group: multiply directly out of PSUM with
            a strided 2-wide reciprocal of the row-sums (no staging copy)."""
            outfin = dbls.tile([P, OWN_T, D], F32, name="outfin", tag="outfin")
            H = OWN_T // 2
            for g2 in range(OWN_T // 2):
                oa = psum_o.tile([P, 2, 512], F32, name="oa", tag="oa")
                for jj in range(2):
                    j = g2 * 2 + jj
                    for k in range(2):
                        nc.tensor.matmul(
                            oa[:, jj, 0:DA], lhsT=xT[k][:, j * P:(j + 1) * P],
                            rhs=gsb[:, k, :],
                            start=(k == 0), stop=(k == 1),
                        )
                rcp2 = smalls.tile([P, 2], F32, tag="rcp2")
                nc.vector.reciprocal(rcp2, oa[:, :, D])
                nc.vector.tensor_mul(
                    outfin[:, g2 * 2:g2 * 2 + 2, :], oa[:, :, 0:D],
                    rcp2.unsqueeze(2).to_broadcast([P, 2, D]))
                if g2 == OWN_T // 4 - 1:
                    nc.sync.dma_start(out=ov[:, 0:H, :],
                                      in_=outfin[:, 0:H, :])
            nc.sync.dma_start(out=ov[:, H:, :], in_=outfin[:, H:, :])

        # software pipeline, depth 2: phase 2 of iter i is emitted after the
        # cc launch of iter i+2, so by emission order every wait is already
        # satisfied — no engine queue ever blocks on an in-flight AllReduce
        pending = []
        for _it in range(iters):
            pending.append(pre_cc(_it))
            if len(pending) > 2:
                phase2(*pending.pop(0))
        while pending:
            phase2(*pending.pop(0))
    return nc


def _get_nc(iters: int = 1):
    if iters not in _nc_cache:
        nc = _build_nc(iters)
        orig = nc.to_json_bytes
        nc.to_json_bytes = lambda: _legalize_sync_waits(orig())
        _nc_cache[iters] = nc
    return _nc_cache[iters]


LAST_RESULTS = None  # BassKernelResults of the most recent run (for profiling)


def kernel(tensor: np.ndarray, trace: bool = False, **trace_kwargs) -> np.ndarray:
    x = np.ascontiguousarray(np.asarray(tensor, dtype=np.float32))
    assert x.shape == (N, D)
    nc = _get_nc()
    in_maps = [
        {"x_own": np.ascontiguousarray(x[i * OWN:(i + 1) * OWN])}
        for i in range(NCORES)
    ]
    global LAST_RESULTS
    LAST_RESULTS = run_bass_kernel_spmd(
        nc, in_maps, core_ids=list(range(NCORES)), trace=trace, **trace_kwargs
    )
    return np.concatenate([r["out"] for r in LAST_RESULTS.results], axis=0)
